# revision 5
# baseline (speedup 1.0000x reference)
"""Distributed attention kernel for Trainium2 (8 NeuronCores).

Problem: B=2, L=2048, DIM=1024, H=16 heads, HD=64.
  qkv = x @ Wqkv; q,k = rmsnorm per head (+scales); RoPE(q, k);
  scores = q k^T / sqrt(HD); p = softmax(scores); o = p v;
  out = o @ Wproj + bproj.

Sharding: tensor-parallel over heads -- 2 heads per core. Each core:
  - computes qkv^T for its 2 heads (lhsT = Wqkv columns, rhs = x^T),
  - rmsnorm (sum-of-squares via indicator matmul, broadcast back via
    tiny K=4 matmul), RoPE via host-precomputed coefficient tables
    (head scale and 1/sqrt(HD) folded in),
  - attention in "transposed score" (key-major) layout: st[m, l] = k.q,
    exp WITHOUT max-subtraction (rmsnorm bounds |scores| <= 8), with a
    ones-column appended to v so the softmax denominator falls out of
    the o-matmul as row 64,
  - AllToAll to switch from head-sharded to sequence-sharded,
  - full output projection on its 512-row shard (+bias).
Host concatenates the 8 [1024, 512] column shards and transposes.
"""

import sys

if "/opt/trn_rl_repo" not in sys.path:
    sys.path.insert(0, "/opt/trn_rl_repo")

import numpy as np

B, L, DIM, H, HD = 2, 2048, 1024, 16, 64
NC = 8
HPC = H // NC          # heads per core = 2
BL = B * L             # 4096 flattened rows
CH = 512               # l-chunk size
NCH = BL // CH         # 8 chunks
EPS = 1e-6
THETA = 10000.0
F = 3 * HPC * HD       # 384 qkv features per core

_CACHE = {}


def _rope_tables():
    """cos/sin coefficient tables [64, L] with rotate-half row layout.

    Row d < 32:  c[d] = cos(l*w_d),  s[d] = sin(l*w_d)   (pairs with d+32)
    Row d >= 32: c[d] = cos(l*w_{d-32}), s[d] = sin(l*w_{d-32})
    Used as:
      out1 = q1*c[0:32]  - q2*s[0:32]
      out2 = q2*c[32:64] + q1*s[32:64]
    Head scale vector and (for q) 1/sqrt(HD) are folded in by caller.
    """
    inv_freq = 1.0 / (THETA ** (np.arange(0, HD, 2, dtype=np.float64) / HD))  # [32]
    ang = np.arange(L, dtype=np.float64)[None, :] * inv_freq[:, None]  # [32, L]
    c, s = np.cos(ang), np.sin(ang)
    return c, s


def _make_tables(scale, fold):
    """Build [64, L] f32 cos/sin tables with per-feature scale folded in.

    out1[d] = q1[d]*(cos*scale[d]*fold)      - q2[d]*(sin*scale[d+32]*fold)
    out2[d+32] = q2[d+32]*(cos*scale[d+32]*fold) + q1[d]*(sin*scale[d]*fold)
    """
    c, s = _rope_tables()
    ct = np.empty((HD, L), np.float64)
    st = np.empty((HD, L), np.float64)
    ct[0:32] = c * (scale[0:32, None] * fold)
    ct[32:64] = c * (scale[32:64, None] * fold)
    st[0:32] = s * (scale[32:64, None] * fold)   # multiplies q2 in out1
    st[32:64] = s * (scale[0:32, None] * fold)   # multiplies q1 in out2
    return ct.astype(np.float32), st.astype(np.float32)


def _host_inputs(x, Wqkv, q_scale, k_scale, Wproj, bproj):
    """Build the per-core input maps."""
    x2 = np.ascontiguousarray(np.asarray(x, np.float32).reshape(BL, DIM))
    xT = np.ascontiguousarray(x2.T)                       # [DIM, BL]
    Wqkv = np.asarray(Wqkv, np.float32)
    Wq = Wqkv[:, 0 * DIM:1 * DIM].reshape(DIM, H, HD)
    Wk = Wqkv[:, 1 * DIM:2 * DIM].reshape(DIM, H, HD)
    Wv = Wqkv[:, 2 * DIM:3 * DIM].reshape(DIM, H, HD)

    qc, qs = _make_tables(np.asarray(q_scale, np.float64), 1.0 / np.sqrt(HD))
    kc, ks = _make_tables(np.asarray(k_scale, np.float64), 1.0)

    ind2col = np.zeros((128, 2), np.float32)
    ind2col[0:64, 0] = 1.0
    ind2col[64:128, 1] = 1.0
    # 8.0 = sqrt(HD): folds the /HD of the mean-square into the
    # broadcast so the device computes 8/sqrt(ssq) = 1/sqrt(ssq/64).
    indbc = np.zeros((2, 128), np.float32)
    indbc[0, 0:64] = 8.0
    indbc[1, 64:128] = 8.0
    ones64 = np.ones((1, 64), np.float32)
    ident = np.eye(128, dtype=np.float32)
    wp = np.ascontiguousarray(np.asarray(Wproj, np.float32))
    bp = np.ascontiguousarray(
        np.asarray(bproj, np.float32).reshape(8, 128).T)    # [128, 8]

    shared = dict(xT=xT, qc=qc, qs=qs, kc=kc, ks=ks, ind2col=ind2col,
                  indbc=indbc, ones64=ones64, ident=ident, wp=wp, bp=bp)
    in_maps = []
    for c in range(NC):
        hA, hB = HPC * c, HPC * c + 1
        wq = np.concatenate(
            [Wq[:, hA], Wq[:, hB], Wk[:, hA], Wk[:, hB], Wv[:, hA], Wv[:, hB]],
            axis=1)                                        # [DIM, 384]
        m = dict(shared)
        m["wq"] = np.ascontiguousarray(wq)
        in_maps.append(m)
    return in_maps


def _build():
    import concourse.bass as bass  # noqa: F401
    import concourse.mybir as mybir
    import concourse.tile as tile
    from concourse import bacc

    fp32 = mybir.dt.float32
    bf16 = mybir.dt.bfloat16
    AF = mybir.ActivationFunctionType

    nc = bacc.Bacc("TRN2", target_bir_lowering=False, debug=False,
                   num_devices=NC)

    xT = nc.dram_tensor("xT", [DIM, BL], fp32, kind="ExternalInput")
    wq = nc.dram_tensor("wq", [DIM, F], fp32, kind="ExternalInput")
    qc = nc.dram_tensor("qc", [HD, L], fp32, kind="ExternalInput")
    qs = nc.dram_tensor("qs", [HD, L], fp32, kind="ExternalInput")
    kc = nc.dram_tensor("kc", [HD, L], fp32, kind="ExternalInput")
    ks = nc.dram_tensor("ks", [HD, L], fp32, kind="ExternalInput")
    ind2col_d = nc.dram_tensor("ind2col", [128, 2], fp32, kind="ExternalInput")
    indbc_d = nc.dram_tensor("indbc", [2, 128], fp32, kind="ExternalInput")
    ones64_d = nc.dram_tensor("ones64", [1, 64], fp32, kind="ExternalInput")
    ident_d = nc.dram_tensor("ident", [128, 128], fp32, kind="ExternalInput")
    wp_d = nc.dram_tensor("wp", [DIM, DIM], fp32, kind="ExternalInput")
    bp_d = nc.dram_tensor("bp", [128, 8], fp32, kind="ExternalInput")
    out_d = nc.dram_tensor("out", [DIM, CH], fp32, kind="ExternalOutput")

    with tile.TileContext(nc) as tc:
        # ---------- static SBUF ----------
        with (
            tc.tile_pool(name="consts", bufs=1) as consts,
            tc.tile_pool(name="wqp", bufs=1) as wqp,
            tc.tile_pool(name="tabs", bufs=1) as tabs,
            tc.tile_pool(name="qkv_sb", bufs=1) as qkv_sb,
            tc.tile_pool(name="wppool", bufs=1) as wppool,
            tc.tile_pool(name="dram", bufs=1, space="DRAM") as dram,
        ):
            ind2col = consts.tile([128, 2], fp32)
            nc.sync.dma_start(ind2col[:], ind2col_d[:])
            indbc = consts.tile([2, 128], fp32)
            nc.sync.dma_start(indbc[:], indbc_d[:])
            ones64 = consts.tile([1, 64], fp32)
            nc.sync.dma_start(ones64[:], ones64_d[:])
            ident = consts.tile([128, 128], fp32)
            nc.sync.dma_start(ident[:], ident_d[:])
            bp_sb = consts.tile([128, 8], fp32)
            nc.sync.dma_start(bp_sb[:], bp_d[:])

            qc_sb = tabs.tile([HD, L], fp32)
            nc.sync.dma_start(qc_sb[:], qc[:])
            qs_sb = tabs.tile([HD, L], fp32)
            nc.sync.dma_start(qs_sb[:], qs[:])
            kc_sb = tabs.tile([HD, L], fp32)
            nc.sync.dma_start(kc_sb[:], kc[:])
            ks_sb = tabs.tile([HD, L], fp32)
            nc.sync.dma_start(ks_sb[:], ks[:])

            wq_sb = []
            for kk in range(8):
                t = wqp.tile([128, F], fp32, name=f"wq{kk}")
                nc.sync.dma_start(t[:], wq[128 * kk:128 * (kk + 1), :])
                wq_sb.append(t)

            # persistent activations
            qTn = qkv_sb.tile([128, BL], bf16)     # 2 heads x 64 rows
            kTn = qkv_sb.tile([128, BL], bf16)
            v_sb = qkv_sb.tile([128, 32 * 130], bf16)  # per m-tile [128,130]
            # rows: head A / head B; cols 0:BL = q, BL:2*BL = k
            inv2 = qkv_sb.tile([2, 2 * BL], fp32)

            a2a_in = dram.tile([DIM, CH], fp32, name="a2a_in")
            a2a_out = dram.tile([DIM, CH], fp32, name="a2a_out")

            # ones columns for the denominator trick (cols 64 and 129 of
            # each 130-wide v block); set everything to 1, data overwrites.
            nc.gpsimd.memset(v_sb[:], 1.0)

            # ---------- phase 1: qkv + rmsnorm + rope ----------
            with (
                tc.tile_pool(name="xt", bufs=10) as xtp,
                tc.tile_pool(name="ps", bufs=3, space="PSUM") as ps,
                tc.tile_pool(name="pb", bufs=2, space="PSUM") as pb,
                tc.tile_pool(name="pi", bufs=2, space="PSUM") as pi,
                tc.tile_pool(name="sqp", bufs=2) as sqp,
                tc.tile_pool(name="sdp", bufs=2) as sdp,
                tc.tile_pool(name="tmp", bufs=6) as tmpp,
                tc.tile_pool(name="tmp2", bufs=4) as tmp2p,
                tc.tile_pool(name="vt", bufs=2) as vtp,
            ):
                for ch in range(NCH):
                    c0 = CH * ch
                    lsl = slice(CH * (ch % 4), CH * (ch % 4) + CH)  # pos in seq
                    xt = []
                    for kk in range(8):
                        t = xtp.tile([128, CH], fp32, tag="xt")
                        nc.sync.dma_start(
                            t[:], xT[128 * kk:128 * (kk + 1), c0:c0 + CH])
                        xt.append(t)
                    pst = []
                    for tix in range(3):
                        p = ps.tile([128, CH], fp32, tag="ps")
                        for kk in range(8):
                            nc.tensor.matmul(
                                p[:], wq_sb[kk][:, 128 * tix:128 * (tix + 1)],
                                xt[kk][:], start=(kk == 0), stop=(kk == 7))
                        pst.append(p)

                    # sum-of-squares -> inv rms for q and k
                    for tix in range(2):
                        sq = sqp.tile([128, CH], fp32, tag="sq")
                        nc.scalar.activation(sq[:], pst[tix][:], AF.Square)
                        ssq = pb.tile([2, CH], fp32, tag="pb")
                        nc.tensor.matmul(ssq[:], ind2col[:], sq[:],
                                         start=True, stop=True)
                        sd = sdp.tile([2, CH], fp32, tag="sd")
                        nc.scalar.activation(sd[:], ssq[:], AF.Sqrt)
                        nc.vector.reciprocal(
                            inv2[:, BL * tix + c0:BL * tix + c0 + CH], sd[:])

                    invbq = pi.tile([128, CH], fp32, tag="pi")
                    nc.tensor.matmul(invbq[:], indbc[:],
                                     inv2[:, c0:c0 + CH], start=True, stop=True)
                    invbk = pi.tile([128, CH], fp32, tag="pi")
                    nc.tensor.matmul(invbk[:], indbc[:],
                                     inv2[:, BL + c0:BL + c0 + CH],
                                     start=True, stop=True)

                    # rope + normalize; q then k
                    for tix, (ct, stb, invb, dst) in enumerate(
                            [(qc_sb, qs_sb, invbq, qTn),
                             (kc_sb, ks_sb, invbk, kTn)]):
                        src = pst[tix]
                        for h in range(2):
                            r0 = 64 * h
                            q1 = src[r0:r0 + 32, :]
                            q2 = src[r0 + 32:r0 + 64, :]
                            t1 = tmpp.tile([32, CH], fp32, tag="t")
                            nc.vector.tensor_mul(t1[:], q1, ct[0:32, lsl])
                            t2 = tmpp.tile([32, CH], fp32, tag="t")
                            nc.vector.tensor_mul(t2[:], q2, stb[0:32, lsl])
                            o12 = tmp2p.tile([64, CH], fp32, tag="o12")
                            nc.vector.tensor_sub(o12[0:32, :], t1[:], t2[:])
                            t3 = tmpp.tile([32, CH], fp32, tag="t")
                            nc.vector.tensor_mul(t3[:], q2, ct[32:64, lsl])
                            t4 = tmpp.tile([32, CH], fp32, tag="t")
                            nc.vector.tensor_mul(t4[:], q1, stb[32:64, lsl])
                            nc.vector.tensor_add(o12[32:64, :], t3[:], t4[:])
                            nc.vector.tensor_mul(
                                dst[r0:r0 + 64, c0:c0 + CH], o12[:],
                                invb[r0:r0 + 64, :])

                    # v: transpose to row-major and install next to ones cols
                    vt = vtp.tile([128, CH], fp32, tag="vt")
                    nc.vector.tensor_copy(vt[:], pst[2][:])
                    tp = pb.tile([128, CH], fp32, tag="pb")
                    for blk in range(4):
                        nc.tensor.transpose(
                            tp[:, 128 * blk:128 * (blk + 1)],
                            vt[:, 128 * blk:128 * (blk + 1)], ident[:])
                    for blk in range(4):
                        mt = 4 * ch + blk
                        nc.vector.tensor_copy(
                            v_sb[:, 130 * mt:130 * mt + 64],
                            tp[:, 128 * blk:128 * blk + 64])
                        nc.vector.tensor_copy(
                            v_sb[:, 130 * mt + 65:130 * mt + 129],
                            tp[:, 128 * blk + 64:128 * (blk + 1)])

            # ---------- wproj load (overlaps attention) ----------
            wp_sb = []
            for ff in range(8):
                t = wppool.tile([128, DIM], fp32, name=f"wp{ff}")
                nc.sync.dma_start(t[:], wp_d[128 * ff:128 * (ff + 1), :])
                wp_sb.append(t)

            # ---------- phase 2: attention ----------
            with (
                tc.tile_pool(name="stp", bufs=3, space="PSUM") as stp,
                tc.tile_pool(name="po", bufs=4, space="PSUM") as pop,
                tc.tile_pool(name="rb", bufs=1, space="PSUM") as rbp,
                tc.tile_pool(name="ptp", bufs=6) as ptp,
                tc.tile_pool(name="rcp", bufs=2) as rcp,
                tc.tile_pool(name="otp", bufs=2) as otp,
            ):
                for bh in range(4):
                    b, h = bh // 2, bh % 2
                    base = L * b
                    hr = slice(64 * h, 64 * h + 64)
                    po_l = [pop.tile([65, CH], fp32, tag="po",
                                     name=f"po{bh}_{i}")
                            for i in range(4)]
                    for m in range(16):
                        moff = base + 128 * m
                        mt = 16 * b + m
                        vsl = v_sb[:, 130 * mt + 65 * h:130 * mt + 65 * h + 65]
                        for li in range(4):
                            loff = base + CH * li
                            st = stp.tile([128, CH], fp32, tag="st")
                            nc.tensor.matmul(
                                st[:], kTn[hr, moff:moff + 128],
                                qTn[hr, loff:loff + CH],
                                start=True, stop=True)
                            pt = ptp.tile([128, CH], bf16, tag="pt")
                            nc.scalar.activation(pt[:], st[:], AF.Exp)
                            nc.tensor.matmul(
                                po_l[li][:], vsl, pt[:],
                                start=(m == 0), stop=(m == 15))
                    for li in range(4):
                        rc = rcp.tile([1, CH], fp32, tag="rc")
                        nc.vector.reciprocal(rc[:], po_l[li][64:65, :])
                        rb = rbp.tile([64, CH], fp32, tag="rb")
                        nc.tensor.matmul(rb[:], ones64[:], rc[:],
                                         start=True, stop=True)
                        rbs = rcp.tile([64, CH], fp32, tag="rbs")
                        nc.scalar.activation(rbs[:], rb[:], AF.Copy)
                        ot = otp.tile([64, CH], fp32, tag="ot")
                        nc.vector.tensor_mul(ot[:], po_l[li][0:64, :], rbs[:])
                        cj = 4 * b + li
                        nc.sync.dma_start(
                            a2a_in[128 * cj + 64 * h:128 * cj + 64 * h + 64, :],
                            ot[:])

            # ---------- phase 3: all-to-all ----------
            nc.gpsimd.collective_compute(
                "AllToAll", mybir.AluOpType.bypass,
                replica_groups=[list(range(NC))],
                ins=[a2a_in[:]],
                outs=[a2a_out[:]],
            )

            # ---------- phase 4: output projection ----------
            with (
                tc.tile_pool(name="ofp", bufs=8) as ofp,
                tc.tile_pool(name="prp", bufs=2, space="PSUM") as prp,
                tc.tile_pool(name="obp", bufs=2) as obp,
            ):
                of = []
                for ff in range(8):
                    t = ofp.tile([128, CH], fp32, tag="of")
                    nc.sync.dma_start(
                        t[:], a2a_out[128 * ff:128 * (ff + 1), :])
                    of.append(t)
                for d in range(8):
                    pr = prp.tile([128, CH], fp32, tag="pr")
                    for ff in range(8):
                        nc.tensor.matmul(
                            pr[:], wp_sb[ff][:, 128 * d:128 * (d + 1)],
                            of[ff][:], start=(ff == 0), stop=(ff == 7))
                    ob = obp.tile([128, CH], fp32, tag="ob")
                    nc.vector.tensor_scalar_add(ob[:], pr[:],
                                                bp_sb[:, d:d + 1])
                    nc.sync.dma_start(out_d[128 * d:128 * (d + 1), :], ob[:])

    nc.compile()
    return nc


def _run(inputs, trace=False, trace_kwargs=None):
    from concourse.bass_utils import run_bass_kernel_spmd

    if "nc" not in _CACHE:
        _CACHE["nc"] = _build()
    nc = _CACHE["nc"]
    in_maps = _host_inputs(**inputs)
    res = run_bass_kernel_spmd(
        nc, in_maps, core_ids=list(range(NC)), trace=trace,
        **(trace_kwargs or {}))
    return res


def kernel(x, Wqkv, q_scale, k_scale, Wproj, bproj):
    res = _run(dict(x=x, Wqkv=Wqkv, q_scale=q_scale, k_scale=k_scale,
                    Wproj=Wproj, bproj=bproj))
    outT = np.concatenate([res.results[c]["out"] for c in range(NC)], axis=1)
    return np.ascontiguousarray(outT.T).reshape(B, L, DIM).astype(np.float32)


if __name__ == "__main__":
    rng = np.random.default_rng(0)
    x = rng.standard_normal((B, L, DIM), dtype=np.float32)
    Wqkv_ = rng.standard_normal((DIM, 3 * DIM), dtype=np.float32) * DIM ** -0.5
    Wproj_ = rng.standard_normal((DIM, DIM), dtype=np.float32) * DIM ** -0.5
    out = kernel(x=x, Wqkv=Wqkv_, q_scale=np.ones(HD, np.float32),
                 k_scale=np.ones(HD, np.float32), Wproj=Wproj_,
                 bproj=np.zeros(DIM, np.float32))
    print(out.shape, out.dtype)


# revision 9
# speedup vs baseline: 1.5293x; 1.5293x over previous
"""Distributed attention kernel for Trainium2 (8 NeuronCores).

Problem: B=2, L=2048, DIM=1024, H=16 heads, HD=64.
  qkv = x @ Wqkv; q,k = rmsnorm per head (+scales); RoPE(q, k);
  scores = q k^T / sqrt(HD); p = softmax(scores); o = p v;
  out = o @ Wproj + bproj.

Sharding: tensor-parallel over heads -- 2 heads per core. Each core:
  - computes qkv^T for its 2 heads (lhsT = Wqkv columns, rhs = x^T),
    in bf16 (fp32 matmul runs double-pass LOW_HIGH on TRN2),
  - rmsnorm (sum-of-squares via indicator matmul, broadcast back via
    tiny matmul), RoPE via host-precomputed coefficient tables
    (head scale and 1/sqrt(HD) folded in),
  - attention in "transposed score" (key-major) layout: st[m, l] = k.q,
    exp WITHOUT max-subtraction (rmsnorm bounds |scores| <= 8), with a
    ones-column appended to v so the softmax denominator falls out of
    the o-matmul as row 64,
  - per-head AllToAll (bf16) to switch from head-sharded to
    sequence-sharded; the h=0 AllToAll overlaps h=1 attention,
  - full output projection on its 512-row shard (+bias).
Host concatenates the 8 [1024, 512] column shards and transposes.
"""

import sys

if "/opt/trn_rl_repo" not in sys.path:
    sys.path.insert(0, "/opt/trn_rl_repo")

import numpy as np
import ml_dtypes

B, L, DIM, H, HD = 2, 2048, 1024, 16, 64
NC = 8
HPC = H // NC          # heads per core = 2
BL = B * L             # 4096 flattened rows
CH = 512               # l-chunk size
NCH = BL // CH         # 8 chunks
EPS = 1e-6
THETA = 10000.0
F = 3 * HPC * HD       # 384 qkv features per core

BF = ml_dtypes.bfloat16
_CACHE = {}


def _rope_tables():
    inv_freq = 1.0 / (THETA ** (np.arange(0, HD, 2, dtype=np.float64) / HD))  # [32]
    ang = np.arange(L, dtype=np.float64)[None, :] * inv_freq[:, None]  # [32, L]
    return np.cos(ang), np.sin(ang)


def _make_tables(scale, fold):
    """[64, L] f32 cos/sin coefficient tables, per-feature scale folded in.

    Device computes, per head (rows r0..r0+63 of the qkv tile):
      tc = src[0:64] * ct          (one [64] mul)
      ts = src[0:64] * st          (one [64] mul)
      out[0:32]  = tc[0:32]  - ts[32:64]
      out[32:64] = tc[32:64] + ts[0:32]
    which equals rotate-half RoPE with scale/fold applied:
      out[d]    = q[d]*cos_d*s(d)*f    - q[d+32]*sin_d*s(d+32)*f   (d<32)
      out[d+32] = q[d+32]*cos_d*s(d+32)*f + q[d]*sin_d*s(d)*f
    """
    c, s = _rope_tables()
    ct = np.empty((HD, L), np.float64)
    st = np.empty((HD, L), np.float64)
    ct[0:32] = c * (scale[0:32, None] * fold)
    ct[32:64] = c * (scale[32:64, None] * fold)
    # ts rows 0:32 multiply q1 (used in out2): sin * scale[d] * fold
    st[0:32] = s * (scale[0:32, None] * fold)
    # ts rows 32:64 multiply q2 (used in out1): sin * scale[d+32] * fold
    st[32:64] = s * (scale[32:64, None] * fold)
    return ct.astype(np.float32), st.astype(np.float32)


def _host_inputs(x, Wqkv, q_scale, k_scale, Wproj, bproj):
    x2 = np.ascontiguousarray(np.asarray(x, np.float32).reshape(BL, DIM))
    xT = np.ascontiguousarray(x2.T.astype(BF))              # [DIM, BL] bf16
    Wqkv = np.asarray(Wqkv, np.float32)
    Wq = Wqkv[:, 0 * DIM:1 * DIM].reshape(DIM, H, HD)
    Wk = Wqkv[:, 1 * DIM:2 * DIM].reshape(DIM, H, HD)
    Wv = Wqkv[:, 2 * DIM:3 * DIM].reshape(DIM, H, HD)

    qc, qs = _make_tables(np.asarray(q_scale, np.float64), 1.0 / np.sqrt(HD))
    kc, ks = _make_tables(np.asarray(k_scale, np.float64), 1.0)

    ind2col = np.zeros((128, 2), BF)
    ind2col[0:64, 0] = 1.0
    ind2col[64:128, 1] = 1.0
    # 8.0 = sqrt(HD): folds the /HD of the mean-square into the
    # broadcast so the device computes 8/sqrt(ssq) = 1/sqrt(ssq/64).
    indbc = np.zeros((2, 128), np.float32)
    indbc[0, 0:64] = 8.0
    indbc[1, 64:128] = 8.0
    ones64 = np.ones((1, 64), np.float32)
    ident = np.eye(128, dtype=BF)
    wp = np.ascontiguousarray(np.asarray(Wproj, np.float32).astype(BF))
    bp = np.ascontiguousarray(
        np.asarray(bproj, np.float32).reshape(8, 128).T)    # [128, 8]

    shared = dict(xT=xT, qc=qc, qs=qs, kc=kc, ks=ks, ind2col=ind2col,
                  indbc=indbc, ones64=ones64, ident=ident, wp=wp, bp=bp)
    in_maps = []
    for c in range(NC):
        hA, hB = HPC * c, HPC * c + 1
        wqc = np.concatenate(
            [Wq[:, hA], Wq[:, hB], Wk[:, hA], Wk[:, hB], Wv[:, hA], Wv[:, hB]],
            axis=1)                                        # [DIM, 384]
        m = dict(shared)
        m["wq"] = np.ascontiguousarray(wqc.astype(BF))
        in_maps.append(m)
    return in_maps


def _build(taps=False):
    import concourse.bass as bass  # noqa: F401
    import concourse.mybir as mybir
    import concourse.tile as tile
    from concourse import bacc

    fp32 = mybir.dt.float32
    bf16 = mybir.dt.bfloat16
    AF = mybir.ActivationFunctionType

    nc = bacc.Bacc("TRN2", target_bir_lowering=False, debug=False,
                   num_devices=NC)

    xT = nc.dram_tensor("xT", [DIM, BL], bf16, kind="ExternalInput")
    wq = nc.dram_tensor("wq", [DIM, F], bf16, kind="ExternalInput")
    qc = nc.dram_tensor("qc", [HD, L], fp32, kind="ExternalInput")
    qs = nc.dram_tensor("qs", [HD, L], fp32, kind="ExternalInput")
    kc = nc.dram_tensor("kc", [HD, L], fp32, kind="ExternalInput")
    ks = nc.dram_tensor("ks", [HD, L], fp32, kind="ExternalInput")
    ind2col_d = nc.dram_tensor("ind2col", [128, 2], bf16, kind="ExternalInput")
    indbc_d = nc.dram_tensor("indbc", [2, 128], fp32, kind="ExternalInput")
    ones64_d = nc.dram_tensor("ones64", [1, 64], fp32, kind="ExternalInput")
    ident_d = nc.dram_tensor("ident", [128, 128], bf16, kind="ExternalInput")
    wp_d = nc.dram_tensor("wp", [DIM, DIM], bf16, kind="ExternalInput")
    bp_d = nc.dram_tensor("bp", [128, 8], fp32, kind="ExternalInput")
    out_d = nc.dram_tensor("out", [DIM, CH], fp32, kind="ExternalOutput")
    if taps:
        tap_qtn = nc.dram_tensor("tap_qtn", [128, CH], bf16,
                                 kind="ExternalOutput")
        tap_ktn = nc.dram_tensor("tap_ktn", [128, CH], bf16,
                                 kind="ExternalOutput")
        tap_v = nc.dram_tensor("tap_v", [128, 4 * 130], bf16,
                               kind="ExternalOutput")
        tap_inv = nc.dram_tensor("tap_inv", [2, 2 * CH], fp32,
                                 kind="ExternalOutput")
        tap_a2ain = nc.dram_tensor("tap_a2ain", [NC * 64, CH], bf16,
                                   kind="ExternalOutput")
        tap_a2aout = nc.dram_tensor("tap_a2aout", [NC * 64, CH], bf16,
                                    kind="ExternalOutput")

    with tile.TileContext(nc) as tc:
        with (
            tc.tile_pool(name="consts", bufs=1) as consts,
            tc.tile_pool(name="wqp", bufs=1) as wqp,
            tc.tile_pool(name="tabs", bufs=1) as tabs,
            tc.tile_pool(name="qkv_sb", bufs=1) as qkv_sb,
            tc.tile_pool(name="wppool", bufs=1) as wppool,
            tc.tile_pool(name="dram", bufs=1, space="DRAM") as dram,
        ):
            ind2col = consts.tile([128, 2], bf16)
            nc.sync.dma_start(ind2col[:], ind2col_d[:])
            indbc = consts.tile([2, 128], fp32)
            nc.sync.dma_start(indbc[:], indbc_d[:])
            ones64 = consts.tile([1, 64], fp32)
            nc.sync.dma_start(ones64[:], ones64_d[:])
            ident = consts.tile([128, 128], bf16)
            nc.sync.dma_start(ident[:], ident_d[:])
            bp_sb = consts.tile([128, 8], fp32)
            nc.sync.dma_start(bp_sb[:], bp_d[:])

            qc_sb = tabs.tile([HD, L], fp32)
            nc.sync.dma_start(qc_sb[:], qc[:])
            qs_sb = tabs.tile([HD, L], fp32)
            nc.sync.dma_start(qs_sb[:], qs[:])
            kc_sb = tabs.tile([HD, L], fp32)
            nc.sync.dma_start(kc_sb[:], kc[:])
            ks_sb = tabs.tile([HD, L], fp32)
            nc.sync.dma_start(ks_sb[:], ks[:])

            wq_sb = []
            for kk in range(8):
                t = wqp.tile([128, F], bf16, name=f"wq{kk}")
                nc.sync.dma_start(t[:], wq[128 * kk:128 * (kk + 1), :])
                wq_sb.append(t)

            # per-chunk persistent activations (separate tiles so the Tile
            # scheduler sees precise deps and can overlap qkv / attention)
            qTn = [qkv_sb.tile([128, CH], bf16, name=f"qTn{c}")
                   for c in range(NCH)]
            kTn = [qkv_sb.tile([128, CH], bf16, name=f"kTn{c}")
                   for c in range(NCH)]
            # v per chunk: 4 m-tiles of [128, 130] (64 vA | 1 | 64 vB | 1)
            v_sb = [qkv_sb.tile([128, 4 * 130], bf16, name=f"v{c}")
                    for c in range(NCH)]
            # inverse rms rows [2]: cols 0:CH q, CH:2*CH k
            inv2 = [qkv_sb.tile([2, 2 * CH], fp32, name=f"inv{c}")
                    for c in range(NCH)]

            # per-head A2A buffers (chunk j = this core's 64 head-features
            # for destination core j)
            a2a_in = [dram.tile([NC * 64, CH], bf16, name=f"a2a_in{h}")
                      for h in range(2)]
            a2a_out = [dram.tile([NC * 64, CH], bf16, name=f"a2a_out{h}")
                       for h in range(2)]

            for c in range(NCH):
                nc.gpsimd.memset(v_sb[c][:], 1.0)

            # ---------- phase 1: qkv + rmsnorm + rope ----------
            with (
                tc.tile_pool(name="xt", bufs=10) as xtp,
                tc.tile_pool(name="ps", bufs=3, space="PSUM") as ps,
                tc.tile_pool(name="pb", bufs=2, space="PSUM") as pb,
                tc.tile_pool(name="pi", bufs=2, space="PSUM") as pi,
                tc.tile_pool(name="sqp", bufs=2) as sqp,
                tc.tile_pool(name="sdp", bufs=2) as sdp,
                tc.tile_pool(name="tmp", bufs=8) as tmpp,
                tc.tile_pool(name="vt", bufs=2) as vtp,
            ):
                for ch in range(NCH):
                    c0 = CH * ch
                    lsl = slice(CH * (ch % 4), CH * (ch % 4) + CH)
                    xt = []
                    for kk in range(8):
                        t = xtp.tile([128, CH], bf16, tag="xt")
                        nc.sync.dma_start(
                            t[:], xT[128 * kk:128 * (kk + 1), c0:c0 + CH])
                        xt.append(t)
                    pst = []
                    for tix in range(3):
                        p = ps.tile([128, CH], fp32, tag="ps")
                        for kk in range(8):
                            nc.tensor.matmul(
                                p[:], wq_sb[kk][:, 128 * tix:128 * (tix + 1)],
                                xt[kk][:], start=(kk == 0), stop=(kk == 7))
                        pst.append(p)

                    # sum-of-squares -> inv rms for q and k
                    for tix in range(2):
                        sq = sqp.tile([128, CH], bf16, tag="sq")
                        nc.scalar.activation(sq[:], pst[tix][:], AF.Square)
                        ssq = pb.tile([2, CH], fp32, tag="pb")
                        nc.tensor.matmul(ssq[:], ind2col[:], sq[:],
                                         start=True, stop=True)
                        sd = sdp.tile([2, CH], fp32, tag="sd")
                        nc.scalar.activation(sd[:], ssq[:], AF.Sqrt)
                        nc.vector.reciprocal_approx_fast(
                            inv2[ch][:, CH * tix:CH * tix + CH], sd[:])

                    invbq = pi.tile([128, CH], fp32, tag="pi")
                    nc.tensor.matmul(invbq[:], indbc[:],
                                     inv2[ch][:, 0:CH], start=True, stop=True)
                    invbk = pi.tile([128, CH], fp32, tag="pi")
                    nc.tensor.matmul(invbk[:], indbc[:],
                                     inv2[ch][:, CH:2 * CH],
                                     start=True, stop=True)

                    # rope + normalize; q then k
                    for tix, (ct, stb, invb, dst) in enumerate(
                            [(qc_sb, qs_sb, invbq, qTn[ch]),
                             (kc_sb, ks_sb, invbk, kTn[ch])]):
                        src = pst[tix]
                        for h in range(2):
                            r0 = 64 * h
                            s64 = src[r0:r0 + 64, :]
                            tc_ = tmpp.tile([64, CH], fp32, tag="tc")
                            nc.vector.tensor_mul(tc_[:], s64, ct[:, lsl])
                            # ts pre-swapped (in0 is PSUM, so the mixed
                            # base-partition access is legal)
                            ts_ = tmpp.tile([64, CH], fp32, tag="ts")
                            nc.vector.tensor_mul(
                                ts_[0:32, :], src[r0 + 32:r0 + 64, :],
                                stb[32:64, lsl])
                            nc.vector.tensor_mul(
                                ts_[32:64, :], src[r0:r0 + 32, :],
                                stb[0:32, lsl])
                            o12 = tmpp.tile([64, CH], fp32, tag="o12")
                            nc.vector.tensor_sub(o12[0:32, :], tc_[0:32, :],
                                                 ts_[0:32, :])
                            nc.vector.tensor_add(o12[32:64, :], tc_[32:64, :],
                                                 ts_[32:64, :])
                            nc.vector.tensor_mul(
                                dst[r0:r0 + 64, :], o12[:],
                                invb[r0:r0 + 64, :])

                    # v: transpose to row-major next to the ones columns
                    vt = vtp.tile([128, CH], bf16, tag="vt")
                    nc.vector.tensor_copy(vt[:], pst[2][:])
                    tp = pb.tile([128, CH], bf16, tag="pb")
                    for blk in range(4):
                        nc.tensor.transpose(
                            tp[:, 128 * blk:128 * (blk + 1)],
                            vt[:, 128 * blk:128 * (blk + 1)], ident[:])
                    for blk in range(4):
                        nc.vector.tensor_copy(
                            v_sb[ch][:, 130 * blk:130 * blk + 64],
                            tp[:, 128 * blk:128 * blk + 64])
                        nc.vector.tensor_copy(
                            v_sb[ch][:, 130 * blk + 65:130 * blk + 129],
                            tp[:, 128 * blk + 64:128 * (blk + 1)])

            if taps:
                nc.sync.dma_start(tap_qtn[:], qTn[0][:])
                nc.sync.dma_start(tap_ktn[:], kTn[0][:])
                nc.sync.dma_start(tap_v[:], v_sb[0][:])
                nc.sync.dma_start(tap_inv[:], inv2[0][:])

            # ---------- wproj load (overlaps attention) ----------
            wp_sb = []
            for ff in range(8):
                t = wppool.tile([128, DIM], bf16, name=f"wp{ff}")
                nc.sync.dma_start(t[:], wp_d[128 * ff:128 * (ff + 1), :])
                wp_sb.append(t)

            # ---------- phase 2: attention (h-major; per-h AllToAll) ------
            with (
                tc.tile_pool(name="stp", bufs=3, space="PSUM") as stp,
                tc.tile_pool(name="po", bufs=4, space="PSUM") as pop,
                tc.tile_pool(name="rb", bufs=1, space="PSUM") as rbp,
                tc.tile_pool(name="ptp", bufs=6) as ptp,
                tc.tile_pool(name="rcp", bufs=2) as rcp,
                tc.tile_pool(name="otp", bufs=2) as otp,
            ):
                for h in range(2):
                    hr = slice(64 * h, 64 * h + 64)
                    for b in range(2):
                        po_l = [pop.tile([65, CH], fp32, tag="po",
                                         name=f"po{h}{b}_{i}")
                                for i in range(4)]
                        for m in range(16):
                            cm = 4 * b + m // 4        # chunk holding m-tile
                            mo = 128 * (m % 4)         # m-tile offset in chunk
                            vo = 130 * (m % 4)
                            vsl = v_sb[cm][:, vo + 65 * h:vo + 65 * h + 65]
                            for li in range(4):
                                cl = 4 * b + li        # chunk holding l-tile
                                st = stp.tile([128, CH], fp32, tag="st")
                                nc.tensor.matmul(
                                    st[:], kTn[cm][hr, mo:mo + 128],
                                    qTn[cl][hr, :],
                                    start=True, stop=True)
                                pt = ptp.tile([128, CH], bf16, tag="pt")
                                nc.scalar.activation(pt[:], st[:], AF.Exp)
                                nc.tensor.matmul(
                                    po_l[li][:], vsl, pt[:],
                                    start=(m == 0), stop=(m == 15))
                        for li in range(4):
                            rc0 = rcp.tile([1, CH], fp32, tag="rc0")
                            nc.scalar.activation(rc0[:], po_l[li][64:65, :],
                                                 AF.Copy)
                            rc = rcp.tile([1, CH], fp32, tag="rc")
                            nc.vector.reciprocal_approx_fast(rc[:], rc0[:])
                            rb = rbp.tile([64, CH], fp32, tag="rb")
                            nc.tensor.matmul(rb[:], ones64[:], rc[:],
                                             start=True, stop=True)
                            rbs = rcp.tile([64, CH], fp32, tag="rbs")
                            nc.scalar.activation(rbs[:], rb[:], AF.Copy)
                            ot = otp.tile([64, CH], bf16, tag="ot")
                            nc.vector.tensor_mul(ot[:], po_l[li][0:64, :],
                                                 rbs[:])
                            cj = 4 * b + li
                            nc.sync.dma_start(
                                a2a_in[h][64 * cj:64 * cj + 64, :], ot[:])
                    # h's AllToAll: h=0's overlaps h=1 attention compute
                    nc.gpsimd.collective_compute(
                        "AllToAll", mybir.AluOpType.bypass,
                        replica_groups=[list(range(NC))],
                        ins=[a2a_in[h][:]],
                        outs=[a2a_out[h][:]],
                    )

            if taps:
                nc.sync.dma_start(tap_a2ain[:], a2a_in[0][:])
                nc.sync.dma_start(tap_a2aout[:], a2a_out[0][:])

            # ---------- phase 3: output projection ----------
            with (
                tc.tile_pool(name="ofp", bufs=8) as ofp,
                tc.tile_pool(name="prp", bufs=2, space="PSUM") as prp,
                tc.tile_pool(name="obp", bufs=2) as obp,
            ):
                of = []
                for ff in range(8):
                    t = ofp.tile([128, CH], bf16, tag="of", name=f"of{ff}")
                    nc.sync.dma_start(
                        t[0:64, :], a2a_out[0][64 * ff:64 * (ff + 1), :])
                    nc.sync.dma_start(
                        t[64:128, :], a2a_out[1][64 * ff:64 * (ff + 1), :])
                    of.append(t)
                for d in range(8):
                    pr = prp.tile([128, CH], fp32, tag="pr")
                    for ff in range(8):
                        nc.tensor.matmul(
                            pr[:], wp_sb[ff][:, 128 * d:128 * (d + 1)],
                            of[ff][:], start=(ff == 0), stop=(ff == 7))
                    ob = obp.tile([128, CH], fp32, tag="ob")
                    nc.vector.tensor_scalar_add(ob[:], pr[:],
                                                bp_sb[:, d:d + 1])
                    nc.sync.dma_start(out_d[128 * d:128 * (d + 1), :], ob[:])

    nc.compile()
    return nc


def _run(inputs, trace=False, trace_kwargs=None):
    from concourse.bass_utils import run_bass_kernel_spmd

    if "nc" not in _CACHE:
        _CACHE["nc"] = _build()
    nc = _CACHE["nc"]
    in_maps = _host_inputs(**inputs)
    res = run_bass_kernel_spmd(
        nc, in_maps, core_ids=list(range(NC)), trace=trace,
        **(trace_kwargs or {}))
    return res


def kernel(x, Wqkv, q_scale, k_scale, Wproj, bproj):
    res = _run(dict(x=x, Wqkv=Wqkv, q_scale=q_scale, k_scale=k_scale,
                    Wproj=Wproj, bproj=bproj))
    outT = np.concatenate([res.results[c]["out"] for c in range(NC)], axis=1)
    return np.ascontiguousarray(outT.T).reshape(B, L, DIM).astype(np.float32)


if __name__ == "__main__":
    rng = np.random.default_rng(0)
    x = rng.standard_normal((B, L, DIM), dtype=np.float32)
    Wqkv_ = rng.standard_normal((DIM, 3 * DIM), dtype=np.float32) * DIM ** -0.5
    Wproj_ = rng.standard_normal((DIM, DIM), dtype=np.float32) * DIM ** -0.5
    out = kernel(x=x, Wqkv=Wqkv_, q_scale=np.ones(HD, np.float32),
                 k_scale=np.ones(HD, np.float32), Wproj=Wproj_,
                 bproj=np.zeros(DIM, np.float32))
    print(out.shape, out.dtype)


# revision 11
# speedup vs baseline: 1.6370x; 1.0705x over previous
"""Distributed attention kernel for Trainium2 (8 NeuronCores).

Problem: B=2, L=2048, DIM=1024, H=16 heads, HD=64.
  qkv = x @ Wqkv; q,k = rmsnorm per head (+scales); RoPE(q, k);
  scores = q k^T / sqrt(HD); p = softmax(scores); o = p v;
  out = o @ Wproj + bproj.

Sharding: tensor-parallel over heads -- 2 heads per core. Each core:
  - computes qkv^T for its 2 heads (lhsT = Wqkv columns, rhs = x^T),
    in bf16 (fp32 matmul runs double-pass LOW_HIGH on TRN2),
  - rmsnorm (sum-of-squares via indicator matmul, broadcast back via
    tiny matmul), RoPE via host-precomputed coefficient tables
    (head scale and 1/sqrt(HD) folded in),
  - attention in "transposed score" (key-major) layout: st[m, l] = k.q,
    exp WITHOUT max-subtraction (rmsnorm bounds |scores| <= 8), with a
    ones-column appended to v so the softmax denominator falls out of
    the o-matmul as row 64,
  - per-head AllToAll (bf16) to switch from head-sharded to
    sequence-sharded; the h=0 AllToAll overlaps h=1 attention,
  - full output projection on its 512-row shard (+bias).
Host concatenates the 8 [1024, 512] column shards and transposes.
"""

import sys

if "/opt/trn_rl_repo" not in sys.path:
    sys.path.insert(0, "/opt/trn_rl_repo")

import numpy as np
import ml_dtypes

B, L, DIM, H, HD = 2, 2048, 1024, 16, 64
NC = 8
HPC = H // NC          # heads per core = 2
BL = B * L             # 4096 flattened rows
CH = 512               # l-chunk size
NCH = BL // CH         # 8 chunks
EPS = 1e-6
THETA = 10000.0
F = 3 * HPC * HD       # 384 qkv features per core

BF = ml_dtypes.bfloat16
_CACHE = {}


def _rope_tables():
    inv_freq = 1.0 / (THETA ** (np.arange(0, HD, 2, dtype=np.float64) / HD))  # [32]
    ang = np.arange(L, dtype=np.float64)[None, :] * inv_freq[:, None]  # [32, L]
    return np.cos(ang), np.sin(ang)


def _make_tables(scale, fold):
    """[64, L] f32 cos/sin coefficient tables, per-feature scale folded in.

    Device computes, per head (rows r0..r0+63 of the qkv tile):
      tc = src[0:64] * ct          (one [64] mul)
      ts = src[0:64] * st          (one [64] mul)
      out[0:32]  = tc[0:32]  - ts[32:64]
      out[32:64] = tc[32:64] + ts[0:32]
    which equals rotate-half RoPE with scale/fold applied:
      out[d]    = q[d]*cos_d*s(d)*f    - q[d+32]*sin_d*s(d+32)*f   (d<32)
      out[d+32] = q[d+32]*cos_d*s(d+32)*f + q[d]*sin_d*s(d)*f
    """
    c, s = _rope_tables()
    ct = np.empty((HD, L), np.float64)
    st = np.empty((HD, L), np.float64)
    ct[0:32] = c * (scale[0:32, None] * fold)
    ct[32:64] = c * (scale[32:64, None] * fold)
    # ts rows 0:32 multiply q1 (used in out2): sin * scale[d] * fold
    st[0:32] = s * (scale[0:32, None] * fold)
    # ts rows 32:64 multiply q2 (used in out1): sin * scale[d+32] * fold
    st[32:64] = s * (scale[32:64, None] * fold)
    return ct.astype(np.float32), st.astype(np.float32)


def _host_inputs(x, Wqkv, q_scale, k_scale, Wproj, bproj):
    x2 = np.ascontiguousarray(np.asarray(x, np.float32).reshape(BL, DIM))
    xT = np.ascontiguousarray(x2.T.astype(BF))              # [DIM, BL] bf16
    Wqkv = np.asarray(Wqkv, np.float32)
    Wq = Wqkv[:, 0 * DIM:1 * DIM].reshape(DIM, H, HD)
    Wk = Wqkv[:, 1 * DIM:2 * DIM].reshape(DIM, H, HD)
    Wv = Wqkv[:, 2 * DIM:3 * DIM].reshape(DIM, H, HD)

    qc, qs = _make_tables(np.asarray(q_scale, np.float64), 1.0 / np.sqrt(HD))
    kc, ks = _make_tables(np.asarray(k_scale, np.float64), 1.0)

    ind2col = np.zeros((128, 2), BF)
    ind2col[0:64, 0] = 1.0
    ind2col[64:128, 1] = 1.0
    # 8.0 = sqrt(HD): folds the /HD of the mean-square into the
    # broadcast so the device computes 8/sqrt(ssq) = 1/sqrt(ssq/64).
    indbc = np.zeros((2, 128), BF)
    indbc[0, 0:64] = 8.0
    indbc[1, 64:128] = 8.0
    ones64 = np.ones((1, 64), BF)
    ident = np.eye(128, dtype=BF)
    wp = np.ascontiguousarray(np.asarray(Wproj, np.float32).astype(BF))
    bp = np.ascontiguousarray(
        np.asarray(bproj, np.float32).reshape(8, 128).T)    # [128, 8]

    shared = dict(xT=xT, qc=qc, qs=qs, kc=kc, ks=ks, ind2col=ind2col,
                  indbc=indbc, ones64=ones64, ident=ident, wp=wp, bp=bp)
    in_maps = []
    for c in range(NC):
        hA, hB = HPC * c, HPC * c + 1
        wqc = np.concatenate(
            [Wq[:, hA], Wq[:, hB], Wk[:, hA], Wk[:, hB], Wv[:, hA], Wv[:, hB]],
            axis=1)                                        # [DIM, 384]
        m = dict(shared)
        m["wq"] = np.ascontiguousarray(wqc.astype(BF))
        in_maps.append(m)
    return in_maps


def _build(taps=False):
    import concourse.bass as bass  # noqa: F401
    import concourse.mybir as mybir
    import concourse.tile as tile
    from concourse import bacc

    fp32 = mybir.dt.float32
    bf16 = mybir.dt.bfloat16
    AF = mybir.ActivationFunctionType

    nc = bacc.Bacc("TRN2", target_bir_lowering=False, debug=False,
                   num_devices=NC)

    xT = nc.dram_tensor("xT", [DIM, BL], bf16, kind="ExternalInput")
    wq = nc.dram_tensor("wq", [DIM, F], bf16, kind="ExternalInput")
    qc = nc.dram_tensor("qc", [HD, L], fp32, kind="ExternalInput")
    qs = nc.dram_tensor("qs", [HD, L], fp32, kind="ExternalInput")
    kc = nc.dram_tensor("kc", [HD, L], fp32, kind="ExternalInput")
    ks = nc.dram_tensor("ks", [HD, L], fp32, kind="ExternalInput")
    ind2col_d = nc.dram_tensor("ind2col", [128, 2], bf16, kind="ExternalInput")
    indbc_d = nc.dram_tensor("indbc", [2, 128], bf16, kind="ExternalInput")
    ones64_d = nc.dram_tensor("ones64", [1, 64], bf16, kind="ExternalInput")
    ident_d = nc.dram_tensor("ident", [128, 128], bf16, kind="ExternalInput")
    wp_d = nc.dram_tensor("wp", [DIM, DIM], bf16, kind="ExternalInput")
    bp_d = nc.dram_tensor("bp", [128, 8], fp32, kind="ExternalInput")
    out_d = nc.dram_tensor("out", [DIM, CH], fp32, kind="ExternalOutput")
    if taps:
        tap_qtn = nc.dram_tensor("tap_qtn", [128, CH], bf16,
                                 kind="ExternalOutput")
        tap_ktn = nc.dram_tensor("tap_ktn", [128, CH], bf16,
                                 kind="ExternalOutput")
        tap_v = nc.dram_tensor("tap_v", [128, 4 * 130], bf16,
                               kind="ExternalOutput")
        tap_a2ain = nc.dram_tensor("tap_a2ain", [NC * 64, CH], bf16,
                                   kind="ExternalOutput")
        tap_a2aout = nc.dram_tensor("tap_a2aout", [NC * 64, CH], bf16,
                                    kind="ExternalOutput")

    with tile.TileContext(nc) as tc:
        with (
            tc.tile_pool(name="consts", bufs=1) as consts,
            tc.tile_pool(name="wqp", bufs=1) as wqp,
            tc.tile_pool(name="tabs", bufs=1) as tabs,
            tc.tile_pool(name="qkv_sb", bufs=1) as qkv_sb,
            tc.tile_pool(name="wppool", bufs=1) as wppool,
            tc.tile_pool(name="dram", bufs=1, space="DRAM") as dram,
        ):
            ind2col = consts.tile([128, 2], bf16)
            nc.sync.dma_start(ind2col[:], ind2col_d[:])
            indbc = consts.tile([2, 128], bf16)
            nc.sync.dma_start(indbc[:], indbc_d[:])
            ones64 = consts.tile([1, 64], bf16)
            nc.sync.dma_start(ones64[:], ones64_d[:])
            ident = consts.tile([128, 128], bf16)
            nc.sync.dma_start(ident[:], ident_d[:])
            bp_sb = consts.tile([128, 8], fp32)
            nc.sync.dma_start(bp_sb[:], bp_d[:])

            qc_sb = tabs.tile([HD, L], fp32)
            nc.sync.dma_start(qc_sb[:], qc[:])
            qs_sb = tabs.tile([HD, L], fp32)
            nc.sync.dma_start(qs_sb[:], qs[:])
            kc_sb = tabs.tile([HD, L], fp32)
            nc.sync.dma_start(kc_sb[:], kc[:])
            ks_sb = tabs.tile([HD, L], fp32)
            nc.sync.dma_start(ks_sb[:], ks[:])

            wq_sb = []
            for kk in range(8):
                t = wqp.tile([128, F], bf16, name=f"wq{kk}")
                nc.sync.dma_start(t[:], wq[128 * kk:128 * (kk + 1), :])
                wq_sb.append(t)

            # per-chunk persistent activations (separate tiles so the Tile
            # scheduler sees precise deps and can overlap qkv / attention)
            qTn = [qkv_sb.tile([128, CH], bf16, name=f"qTn{c}")
                   for c in range(NCH)]
            kTn = [qkv_sb.tile([128, CH], bf16, name=f"kTn{c}")
                   for c in range(NCH)]
            # v per chunk: 4 m-tiles of [128, 130] (64 vA | 1 | 64 vB | 1)
            v_sb = [qkv_sb.tile([128, 4 * 130], bf16, name=f"v{c}")
                    for c in range(NCH)]


            # per-head A2A buffers (chunk j = this core's 64 head-features
            # for destination core j)
            a2a_in = [dram.tile([NC * 64, CH], bf16, name=f"a2a_in{h}")
                      for h in range(2)]
            a2a_out = [dram.tile([NC * 64, CH], bf16, name=f"a2a_out{h}")
                       for h in range(2)]

            for c in range(NCH):
                nc.gpsimd.memset(v_sb[c][:], 1.0)

            # ---------- phase 1: qkv + rmsnorm + rope ----------
            with (
                tc.tile_pool(name="xt", bufs=10) as xtp,
                tc.tile_pool(name="ps", bufs=3, space="PSUM") as ps,
                tc.tile_pool(name="pb", bufs=2, space="PSUM") as pb,
                tc.tile_pool(name="pi", bufs=2, space="PSUM") as pi,
                tc.tile_pool(name="sqp", bufs=2) as sqp,
                tc.tile_pool(name="sdp", bufs=4) as sdp,
                tc.tile_pool(name="tmp", bufs=8) as tmpp,
                tc.tile_pool(name="vt", bufs=2) as vtp,
            ):
                for ch in range(NCH):
                    c0 = CH * ch
                    lsl = slice(CH * (ch % 4), CH * (ch % 4) + CH)
                    xt = []
                    for kk in range(8):
                        t = xtp.tile([128, CH], bf16, tag="xt")
                        nc.sync.dma_start(
                            t[:], xT[128 * kk:128 * (kk + 1), c0:c0 + CH])
                        xt.append(t)
                    pst = []
                    for tix in range(3):
                        p = ps.tile([128, CH], fp32, tag="ps")
                        for kk in range(8):
                            nc.tensor.matmul(
                                p[:], wq_sb[kk][:, 128 * tix:128 * (tix + 1)],
                                xt[kk][:], start=(kk == 0), stop=(kk == 7))
                        pst.append(p)

                    # sum-of-squares -> inv rms for q and k
                    ivbs = []
                    for tix in range(2):
                        sq = sqp.tile([128, CH], bf16, tag="sq")
                        nc.scalar.activation(sq[:], pst[tix][:], AF.Square)
                        ssq = pb.tile([2, CH], fp32, tag="pb")
                        nc.tensor.matmul(ssq[:], ind2col[:], sq[:],
                                         start=True, stop=True)
                        sd = sdp.tile([2, CH], fp32, tag="sd")
                        nc.scalar.activation(sd[:], ssq[:], AF.Sqrt)
                        iv = sdp.tile([2, CH], fp32, tag="iv")
                        nc.vector.reciprocal_approx_fast(iv[:], sd[:])
                        ivb = sdp.tile([2, CH], bf16, tag="ivb",
                                       name=f"ivb{ch}_{tix}")
                        nc.vector.tensor_copy(ivb[:], iv[:])
                        ivbs.append(ivb)

                    invbq = pi.tile([128, CH], fp32, tag="pi")
                    nc.tensor.matmul(invbq[:], indbc[:],
                                     ivbs[0][:], start=True, stop=True)
                    invbk = pi.tile([128, CH], fp32, tag="pi")
                    nc.tensor.matmul(invbk[:], indbc[:],
                                     ivbs[1][:], start=True, stop=True)

                    # rope + normalize; q then k
                    for tix, (ct, stb, invb, dst) in enumerate(
                            [(qc_sb, qs_sb, invbq, qTn[ch]),
                             (kc_sb, ks_sb, invbk, kTn[ch])]):
                        src = pst[tix]
                        for h in range(2):
                            r0 = 64 * h
                            # stage the head in SBUF at base partition 0 so
                            # the DVE ops below hit the fp32 2x SBUF mode
                            stg = tmpp.tile([64, CH], fp32, tag="stg")
                            nc.scalar.activation(stg[:], src[r0:r0 + 64, :],
                                                 AF.Copy)
                            tc_ = tmpp.tile([64, CH], fp32, tag="tc")
                            nc.vector.tensor_mul(tc_[:], stg[:], ct[:, lsl])
                            # ts pre-swapped; input bases match (32/0)
                            ts_ = tmpp.tile([64, CH], fp32, tag="ts")
                            nc.vector.tensor_mul(
                                ts_[0:32, :], stg[32:64, :],
                                stb[32:64, lsl])
                            nc.vector.tensor_mul(
                                ts_[32:64, :], stg[0:32, :],
                                stb[0:32, lsl])
                            o12 = tmpp.tile([64, CH], fp32, tag="o12")
                            nc.vector.tensor_sub(o12[0:32, :], tc_[0:32, :],
                                                 ts_[0:32, :])
                            nc.vector.tensor_add(o12[32:64, :], tc_[32:64, :],
                                                 ts_[32:64, :])
                            nc.vector.tensor_mul(
                                dst[r0:r0 + 64, :], o12[:],
                                invb[r0:r0 + 64, :])

                    # v: transpose to row-major next to the ones columns
                    vt = vtp.tile([128, CH], bf16, tag="vt")
                    nc.vector.tensor_copy(vt[:], pst[2][:])
                    tp = pb.tile([128, CH], bf16, tag="pb")
                    for blk in range(4):
                        nc.tensor.transpose(
                            tp[:, 128 * blk:128 * (blk + 1)],
                            vt[:, 128 * blk:128 * (blk + 1)], ident[:])
                    for blk in range(4):
                        nc.vector.tensor_copy(
                            v_sb[ch][:, 130 * blk:130 * blk + 64],
                            tp[:, 128 * blk:128 * blk + 64])
                        nc.vector.tensor_copy(
                            v_sb[ch][:, 130 * blk + 65:130 * blk + 129],
                            tp[:, 128 * blk + 64:128 * (blk + 1)])

            if taps:
                nc.sync.dma_start(tap_qtn[:], qTn[0][:])
                nc.sync.dma_start(tap_ktn[:], kTn[0][:])
                nc.sync.dma_start(tap_v[:], v_sb[0][:])

            # ---------- wproj load (overlaps attention) ----------
            wp_sb = []
            for ff in range(8):
                t = wppool.tile([128, DIM], bf16, name=f"wp{ff}")
                nc.sync.dma_start(t[:], wp_d[128 * ff:128 * (ff + 1), :])
                wp_sb.append(t)

            # ---------- phase 2: attention (h-major; per-h AllToAll) ------
            with (
                tc.tile_pool(name="stp", bufs=3, space="PSUM") as stp,
                tc.tile_pool(name="po", bufs=4, space="PSUM") as pop,
                tc.tile_pool(name="rb", bufs=1, space="PSUM") as rbp,
                tc.tile_pool(name="ptp", bufs=8) as ptp,
                tc.tile_pool(name="rcp", bufs=2) as rcp,
                tc.tile_pool(name="otp", bufs=2) as otp,
            ):
                for h in range(2):
                    hr = slice(64 * h, 64 * h + 64)
                    for b in range(2):
                        po_l = [pop.tile([65, CH], fp32, tag="po",
                                         name=f"po{h}{b}_{i}")
                                for i in range(4)]
                        for m in range(16):
                            cm = 4 * b + m // 4        # chunk holding m-tile
                            mo = 128 * (m % 4)         # m-tile offset in chunk
                            vo = 130 * (m % 4)
                            vsl = v_sb[cm][:, vo + 65 * h:vo + 65 * h + 65]
                            sts, pts = [], []
                            for li in range(4):
                                cl = 4 * b + li        # chunk holding l-tile
                                st = stp.tile([128, CH], fp32, tag="st")
                                nc.tensor.matmul(
                                    st[:], kTn[cm][hr, mo:mo + 128],
                                    qTn[cl][hr, :],
                                    start=True, stop=True)
                                sts.append(st)
                                pt = ptp.tile([128, CH], bf16, tag="pt")
                                nc.scalar.activation(pt[:], st[:], AF.Exp)
                                pts.append(pt)
                            for li in range(4):
                                nc.tensor.matmul(
                                    po_l[li][:], vsl, pts[li][:],
                                    start=(m == 0), stop=(m == 15))
                        for li in range(4):
                            rc0 = rcp.tile([1, CH], fp32, tag="rc0")
                            nc.scalar.activation(rc0[:], po_l[li][64:65, :],
                                                 AF.Copy)
                            rc = rcp.tile([1, CH], fp32, tag="rc")
                            nc.vector.reciprocal_approx_fast(rc[:], rc0[:])
                            rcb = rcp.tile([1, CH], bf16, tag="rcb")
                            nc.vector.tensor_copy(rcb[:], rc[:])
                            rb = rbp.tile([64, CH], fp32, tag="rb")
                            nc.tensor.matmul(rb[:], ones64[:], rcb[:],
                                             start=True, stop=True)
                            rbs = rcp.tile([64, CH], fp32, tag="rbs")
                            nc.scalar.activation(rbs[:], rb[:], AF.Copy)
                            ot = otp.tile([64, CH], bf16, tag="ot")
                            nc.vector.tensor_mul(ot[:], po_l[li][0:64, :],
                                                 rbs[:])
                            cj = 4 * b + li
                            nc.sync.dma_start(
                                a2a_in[h][64 * cj:64 * cj + 64, :], ot[:])
                    # h's AllToAll: h=0's overlaps h=1 attention compute
                    nc.gpsimd.collective_compute(
                        "AllToAll", mybir.AluOpType.bypass,
                        replica_groups=[list(range(NC))],
                        ins=[a2a_in[h][:]],
                        outs=[a2a_out[h][:]],
                    )

            if taps:
                nc.sync.dma_start(tap_a2ain[:], a2a_in[0][:])
                nc.sync.dma_start(tap_a2aout[:], a2a_out[0][:])

            # ---------- phase 3: output projection ----------
            with (
                tc.tile_pool(name="ofp", bufs=8) as ofp,
                tc.tile_pool(name="prp", bufs=2, space="PSUM") as prp,
                tc.tile_pool(name="obp", bufs=2) as obp,
            ):
                of = []
                for ff in range(8):
                    t = ofp.tile([128, CH], bf16, tag="of", name=f"of{ff}")
                    nc.sync.dma_start(
                        t[0:64, :], a2a_out[0][64 * ff:64 * (ff + 1), :])
                    nc.sync.dma_start(
                        t[64:128, :], a2a_out[1][64 * ff:64 * (ff + 1), :])
                    of.append(t)
                for d in range(8):
                    pr = prp.tile([128, CH], fp32, tag="pr")
                    for ff in range(8):
                        nc.tensor.matmul(
                            pr[:], wp_sb[ff][:, 128 * d:128 * (d + 1)],
                            of[ff][:], start=(ff == 0), stop=(ff == 7))
                    ob = obp.tile([128, CH], fp32, tag="ob")
                    nc.vector.tensor_scalar_add(ob[:], pr[:],
                                                bp_sb[:, d:d + 1])
                    nc.sync.dma_start(out_d[128 * d:128 * (d + 1), :], ob[:])

    nc.compile()
    return nc


def _run(inputs, trace=False, trace_kwargs=None):
    from concourse.bass_utils import run_bass_kernel_spmd

    if "nc" not in _CACHE:
        _CACHE["nc"] = _build()
    nc = _CACHE["nc"]
    in_maps = _host_inputs(**inputs)
    res = run_bass_kernel_spmd(
        nc, in_maps, core_ids=list(range(NC)), trace=trace,
        **(trace_kwargs or {}))
    return res


def kernel(x, Wqkv, q_scale, k_scale, Wproj, bproj):
    res = _run(dict(x=x, Wqkv=Wqkv, q_scale=q_scale, k_scale=k_scale,
                    Wproj=Wproj, bproj=bproj))
    outT = np.concatenate([res.results[c]["out"] for c in range(NC)], axis=1)
    return np.ascontiguousarray(outT.T).reshape(B, L, DIM).astype(np.float32)


if __name__ == "__main__":
    rng = np.random.default_rng(0)
    x = rng.standard_normal((B, L, DIM), dtype=np.float32)
    Wqkv_ = rng.standard_normal((DIM, 3 * DIM), dtype=np.float32) * DIM ** -0.5
    Wproj_ = rng.standard_normal((DIM, DIM), dtype=np.float32) * DIM ** -0.5
    out = kernel(x=x, Wqkv=Wqkv_, q_scale=np.ones(HD, np.float32),
                 k_scale=np.ones(HD, np.float32), Wproj=Wproj_,
                 bproj=np.zeros(DIM, np.float32))
    print(out.shape, out.dtype)


# revision 12
# speedup vs baseline: 1.8509x; 1.1306x over previous
"""Distributed attention kernel for Trainium2 (8 NeuronCores).

Problem: B=2, L=2048, DIM=1024, H=16 heads, HD=64.
  qkv = x @ Wqkv; q,k = rmsnorm per head (+scales); RoPE(q, k);
  scores = q k^T / sqrt(HD); p = softmax(scores); o = p v;
  out = o @ Wproj + bproj.

Sharding: tensor-parallel over heads -- 2 heads per core. Each core:
  - computes qkv^T for its 2 heads (lhsT = Wqkv columns, rhs = x^T),
    in bf16 (fp32 matmul runs double-pass LOW_HIGH on TRN2),
  - rmsnorm (sum-of-squares via indicator matmul, broadcast back via
    tiny matmul), RoPE via host-precomputed coefficient tables
    (head scale and 1/sqrt(HD) folded in),
  - attention in "transposed score" (key-major) layout: st[m, l] = k.q,
    exp WITHOUT max-subtraction (rmsnorm bounds |scores| <= 8), with a
    ones-column appended to v so the softmax denominator falls out of
    the o-matmul as row 64,
  - per-head AllToAll (bf16) to switch from head-sharded to
    sequence-sharded; the h=0 AllToAll overlaps h=1 attention,
  - full output projection on its 512-row shard (+bias).
Host concatenates the 8 [1024, 512] column shards and transposes.
"""

import sys

if "/opt/trn_rl_repo" not in sys.path:
    sys.path.insert(0, "/opt/trn_rl_repo")

import numpy as np
import ml_dtypes

B, L, DIM, H, HD = 2, 2048, 1024, 16, 64
NC = 8
HPC = H // NC          # heads per core = 2
BL = B * L             # 4096 flattened rows
CH = 512               # l-chunk size
NCH = BL // CH         # 8 chunks
EPS = 1e-6
THETA = 10000.0
F = 3 * HPC * HD       # 384 qkv features per core

BF = ml_dtypes.bfloat16
_CACHE = {}


def _rope_tables():
    inv_freq = 1.0 / (THETA ** (np.arange(0, HD, 2, dtype=np.float64) / HD))  # [32]
    ang = np.arange(L, dtype=np.float64)[None, :] * inv_freq[:, None]  # [32, L]
    return np.cos(ang), np.sin(ang)


def _make_tables(scale, fold):
    """[64, L] f32 cos/sin coefficient tables, per-feature scale folded in.

    Device computes, per head (rows r0..r0+63 of the qkv tile):
      tc = src[0:64] * ct          (one [64] mul)
      ts = src[0:64] * st          (one [64] mul)
      out[0:32]  = tc[0:32]  - ts[32:64]
      out[32:64] = tc[32:64] + ts[0:32]
    which equals rotate-half RoPE with scale/fold applied:
      out[d]    = q[d]*cos_d*s(d)*f    - q[d+32]*sin_d*s(d+32)*f   (d<32)
      out[d+32] = q[d+32]*cos_d*s(d+32)*f + q[d]*sin_d*s(d)*f
    """
    c, s = _rope_tables()
    ct = np.empty((HD, L), np.float64)
    st = np.empty((HD, L), np.float64)
    ct[0:32] = c * (scale[0:32, None] * fold)
    ct[32:64] = c * (scale[32:64, None] * fold)
    # ts rows 0:32 multiply q1 (used in out2): sin * scale[d] * fold
    st[0:32] = s * (scale[0:32, None] * fold)
    # ts rows 32:64 multiply q2 (used in out1, subtracted -> sign folded):
    st[32:64] = -s * (scale[32:64, None] * fold)
    return ct.astype(BF), st.astype(BF)


def _host_inputs(x, Wqkv, q_scale, k_scale, Wproj, bproj):
    x2 = np.ascontiguousarray(np.asarray(x, np.float32).reshape(BL, DIM))
    xT = np.ascontiguousarray(x2.T.astype(BF))              # [DIM, BL] bf16
    Wqkv = np.asarray(Wqkv, np.float32)
    Wq = Wqkv[:, 0 * DIM:1 * DIM].reshape(DIM, H, HD)
    Wk = Wqkv[:, 1 * DIM:2 * DIM].reshape(DIM, H, HD)
    Wv = Wqkv[:, 2 * DIM:3 * DIM].reshape(DIM, H, HD)

    qc, qs = _make_tables(np.asarray(q_scale, np.float64), 1.0 / np.sqrt(HD))
    kc, ks = _make_tables(np.asarray(k_scale, np.float64), 1.0)

    ind2col = np.zeros((128, 2), BF)
    ind2col[0:64, 0] = 1.0
    ind2col[64:128, 1] = 1.0
    # 8.0 = sqrt(HD): folds the /HD of the mean-square into the
    # broadcast so the device computes 8/sqrt(ssq) = 1/sqrt(ssq/64).
    indbc = np.zeros((2, 128), BF)
    indbc[0, 0:64] = 8.0
    indbc[1, 64:128] = 8.0
    ones64 = np.ones((1, 64), BF)
    ident = np.eye(128, dtype=BF)
    wp = np.ascontiguousarray(np.asarray(Wproj, np.float32).astype(BF))
    bp = np.ascontiguousarray(
        np.asarray(bproj, np.float32).reshape(8, 128).T)    # [128, 8]

    shared = dict(xT=xT, qc=qc, qs=qs, kc=kc, ks=ks, ind2col=ind2col,
                  indbc=indbc, ones64=ones64, ident=ident, wp=wp, bp=bp)
    in_maps = []
    for c in range(NC):
        hA, hB = HPC * c, HPC * c + 1
        wqc = np.concatenate(
            [Wq[:, hA], Wq[:, hB], Wk[:, hA], Wk[:, hB], Wv[:, hA], Wv[:, hB]],
            axis=1)                                        # [DIM, 384]
        m = dict(shared)
        m["wq"] = np.ascontiguousarray(wqc.astype(BF))
        in_maps.append(m)
    return in_maps


def _build(taps=False):
    import concourse.bass as bass  # noqa: F401
    import concourse.mybir as mybir
    import concourse.tile as tile
    from concourse import bacc

    fp32 = mybir.dt.float32
    bf16 = mybir.dt.bfloat16
    AF = mybir.ActivationFunctionType

    nc = bacc.Bacc("TRN2", target_bir_lowering=False, debug=False,
                   num_devices=NC)

    xT = nc.dram_tensor("xT", [DIM, BL], bf16, kind="ExternalInput")
    wq = nc.dram_tensor("wq", [DIM, F], bf16, kind="ExternalInput")
    qc = nc.dram_tensor("qc", [HD, L], bf16, kind="ExternalInput")
    qs = nc.dram_tensor("qs", [HD, L], bf16, kind="ExternalInput")
    kc = nc.dram_tensor("kc", [HD, L], bf16, kind="ExternalInput")
    ks = nc.dram_tensor("ks", [HD, L], bf16, kind="ExternalInput")
    ind2col_d = nc.dram_tensor("ind2col", [128, 2], bf16, kind="ExternalInput")
    indbc_d = nc.dram_tensor("indbc", [2, 128], bf16, kind="ExternalInput")
    ones64_d = nc.dram_tensor("ones64", [1, 64], bf16, kind="ExternalInput")
    ident_d = nc.dram_tensor("ident", [128, 128], bf16, kind="ExternalInput")
    wp_d = nc.dram_tensor("wp", [DIM, DIM], bf16, kind="ExternalInput")
    bp_d = nc.dram_tensor("bp", [128, 8], fp32, kind="ExternalInput")
    out_d = nc.dram_tensor("out", [DIM, CH], fp32, kind="ExternalOutput")
    if taps:
        tap_qtn = nc.dram_tensor("tap_qtn", [128, CH], bf16,
                                 kind="ExternalOutput")
        tap_ktn = nc.dram_tensor("tap_ktn", [128, CH], bf16,
                                 kind="ExternalOutput")
        tap_v = nc.dram_tensor("tap_v", [128, 4 * 130], bf16,
                               kind="ExternalOutput")
        tap_a2ain = nc.dram_tensor("tap_a2ain", [NC * 64, CH], bf16,
                                   kind="ExternalOutput")
        tap_a2aout = nc.dram_tensor("tap_a2aout", [NC * 64, CH], bf16,
                                    kind="ExternalOutput")

    with tile.TileContext(nc) as tc:
        with (
            tc.tile_pool(name="consts", bufs=1) as consts,
            tc.tile_pool(name="wqp", bufs=1) as wqp,
            tc.tile_pool(name="tabs", bufs=1) as tabs,
            tc.tile_pool(name="qkv_sb", bufs=1) as qkv_sb,
            tc.tile_pool(name="wppool", bufs=1) as wppool,
            tc.tile_pool(name="dram", bufs=1, space="DRAM") as dram,
        ):
            ind2col = consts.tile([128, 2], bf16)
            nc.sync.dma_start(ind2col[:], ind2col_d[:])
            indbc = consts.tile([2, 128], bf16)
            nc.sync.dma_start(indbc[:], indbc_d[:])
            ones64 = consts.tile([1, 64], bf16)
            nc.sync.dma_start(ones64[:], ones64_d[:])
            ident = consts.tile([128, 128], bf16)
            nc.sync.dma_start(ident[:], ident_d[:])
            bp_sb = consts.tile([128, 8], fp32)
            nc.sync.dma_start(bp_sb[:], bp_d[:])

            qc_sb = tabs.tile([HD, L], bf16)
            nc.sync.dma_start(qc_sb[:], qc[:])
            qs_sb = tabs.tile([HD, L], bf16)
            nc.sync.dma_start(qs_sb[:], qs[:])
            kc_sb = tabs.tile([HD, L], bf16)
            nc.sync.dma_start(kc_sb[:], kc[:])
            ks_sb = tabs.tile([HD, L], bf16)
            nc.sync.dma_start(ks_sb[:], ks[:])

            wq_sb = []
            for kk in range(8):
                t = wqp.tile([128, F], bf16, name=f"wq{kk}")
                nc.sync.dma_start(t[:], wq[128 * kk:128 * (kk + 1), :])
                wq_sb.append(t)

            # per-chunk persistent activations (separate tiles so the Tile
            # scheduler sees precise deps and can overlap qkv / attention)
            qTn = [qkv_sb.tile([128, CH], bf16, name=f"qTn{c}")
                   for c in range(NCH)]
            kTn = [qkv_sb.tile([128, CH], bf16, name=f"kTn{c}")
                   for c in range(NCH)]
            # v per chunk: 4 m-tiles of [128, 130] (64 vA | 1 | 64 vB | 1)
            v_sb = [qkv_sb.tile([128, 4 * 130], bf16, name=f"v{c}")
                    for c in range(NCH)]


            # per-head A2A buffers (chunk j = this core's 64 head-features
            # for destination core j)
            a2a_in = [dram.tile([NC * 64, CH], bf16, name=f"a2a_in{h}")
                      for h in range(2)]
            a2a_out = [dram.tile([NC * 64, CH], bf16, name=f"a2a_out{h}")
                       for h in range(2)]

            for c in range(NCH):
                nc.gpsimd.memset(v_sb[c][:], 1.0)

            # ---------- phase 1: qkv + rmsnorm + rope ----------
            with (
                tc.tile_pool(name="xt", bufs=10) as xtp,
                tc.tile_pool(name="ps", bufs=3, space="PSUM") as ps,
                tc.tile_pool(name="pb", bufs=2, space="PSUM") as pb,
                tc.tile_pool(name="pi", bufs=2, space="PSUM") as pi,
                tc.tile_pool(name="sqp", bufs=2) as sqp,
                tc.tile_pool(name="sdp", bufs=4) as sdp,
                tc.tile_pool(name="tmp", bufs=8) as tmpp,
                tc.tile_pool(name="vt", bufs=2) as vtp,
            ):
                for ch in range(NCH):
                    c0 = CH * ch
                    lsl = slice(CH * (ch % 4), CH * (ch % 4) + CH)
                    xt = []
                    for kk in range(8):
                        t = xtp.tile([128, CH], bf16, tag="xt")
                        nc.sync.dma_start(
                            t[:], xT[128 * kk:128 * (kk + 1), c0:c0 + CH])
                        xt.append(t)
                    pst = []
                    for tix in range(3):
                        p = ps.tile([128, CH], fp32, tag="ps")
                        for kk in range(8):
                            nc.tensor.matmul(
                                p[:], wq_sb[kk][:, 128 * tix:128 * (tix + 1)],
                                xt[kk][:], start=(kk == 0), stop=(kk == 7))
                        pst.append(p)

                    # sum-of-squares -> inv rms for q and k
                    ivbs = []
                    for tix in range(2):
                        sq = sqp.tile([128, CH], bf16, tag="sq")
                        nc.scalar.activation(sq[:], pst[tix][:], AF.Square)
                        ssq = pb.tile([2, CH], fp32, tag="pb")
                        nc.tensor.matmul(ssq[:], ind2col[:], sq[:],
                                         start=True, stop=True)
                        sd = sdp.tile([2, CH], fp32, tag="sd")
                        nc.scalar.activation(sd[:], ssq[:], AF.Sqrt)
                        iv = sdp.tile([2, CH], fp32, tag="iv")
                        nc.vector.reciprocal_approx_fast(iv[:], sd[:])
                        ivb = sdp.tile([2, CH], bf16, tag="ivb",
                                       name=f"ivb{ch}_{tix}")
                        nc.vector.tensor_copy(ivb[:], iv[:])
                        ivbs.append(ivb)

                    invbq = pi.tile([128, CH], fp32, tag="pi")
                    nc.tensor.matmul(invbq[:], indbc[:],
                                     ivbs[0][:], start=True, stop=True)
                    invbk = pi.tile([128, CH], fp32, tag="pi")
                    nc.tensor.matmul(invbk[:], indbc[:],
                                     ivbs[1][:], start=True, stop=True)

                    # rope + normalize; q then k
                    for tix, (ct, stb, invb, dst) in enumerate(
                            [(qc_sb, qs_sb, invbq, qTn[ch]),
                             (kc_sb, ks_sb, invbk, kTn[ch])]):
                        src = pst[tix]
                        # broadcast inv to bf16 SBUF so every DVE op below
                        # runs bf16 2x_1P mode
                        invbs = tmpp.tile([128, CH], bf16, tag="invbs")
                        nc.scalar.activation(invbs[:], invb[:], AF.Copy)
                        o12 = tmpp.tile([128, CH], bf16, tag="o12")
                        for h in range(2):
                            r0 = 64 * h
                            stg = tmpp.tile([64, CH], bf16, tag="stg")
                            nc.scalar.activation(stg[:], src[r0:r0 + 64, :],
                                                 AF.Copy)
                            tc_ = tmpp.tile([64, CH], bf16, tag="tc")
                            nc.vector.tensor_mul(tc_[:], stg[:], ct[:, lsl])
                            # ts pre-swapped; sin sign folded into the table
                            ts_ = tmpp.tile([64, CH], bf16, tag="ts")
                            nc.vector.tensor_mul(
                                ts_[0:32, :], stg[32:64, :],
                                stb[32:64, lsl])
                            nc.vector.tensor_mul(
                                ts_[32:64, :], stg[0:32, :],
                                stb[0:32, lsl])
                            nc.vector.tensor_add(o12[r0:r0 + 64, :], tc_[:],
                                                 ts_[:])
                        nc.vector.tensor_mul(dst[:, :], o12[:], invbs[:])

                    # v: transpose to row-major next to the ones columns
                    vt = vtp.tile([128, CH], bf16, tag="vt")
                    nc.vector.tensor_copy(vt[:], pst[2][:])
                    tp = pb.tile([128, CH], bf16, tag="pb")
                    for blk in range(4):
                        nc.tensor.transpose(
                            tp[:, 128 * blk:128 * (blk + 1)],
                            vt[:, 128 * blk:128 * (blk + 1)], ident[:])
                    for blk in range(4):
                        nc.vector.tensor_copy(
                            v_sb[ch][:, 130 * blk:130 * blk + 64],
                            tp[:, 128 * blk:128 * blk + 64])
                        nc.vector.tensor_copy(
                            v_sb[ch][:, 130 * blk + 65:130 * blk + 129],
                            tp[:, 128 * blk + 64:128 * (blk + 1)])

            if taps:
                nc.sync.dma_start(tap_qtn[:], qTn[0][:])
                nc.sync.dma_start(tap_ktn[:], kTn[0][:])
                nc.sync.dma_start(tap_v[:], v_sb[0][:])

            # ---------- wproj load (overlaps attention) ----------
            wp_sb = []
            for ff in range(8):
                t = wppool.tile([128, DIM], bf16, name=f"wp{ff}")
                nc.sync.dma_start(t[:], wp_d[128 * ff:128 * (ff + 1), :])
                wp_sb.append(t)

            # ---------- phase 2: attention (h-major; per-h AllToAll) ------
            with (
                tc.tile_pool(name="stp", bufs=3, space="PSUM") as stp,
                tc.tile_pool(name="po", bufs=4, space="PSUM") as pop,
                tc.tile_pool(name="rb", bufs=1, space="PSUM") as rbp,
                tc.tile_pool(name="ptp", bufs=8) as ptp,
                tc.tile_pool(name="rcp", bufs=2) as rcp,
                tc.tile_pool(name="otp", bufs=2) as otp,
            ):
                for h in range(2):
                    hr = slice(64 * h, 64 * h + 64)
                    for b in range(2):
                        po_l = [pop.tile([65, CH], fp32, tag="po",
                                         name=f"po{h}{b}_{i}")
                                for i in range(4)]
                        for m in range(16):
                            cm = 4 * b + m // 4        # chunk holding m-tile
                            mo = 128 * (m % 4)         # m-tile offset in chunk
                            vo = 130 * (m % 4)
                            vsl = v_sb[cm][:, vo + 65 * h:vo + 65 * h + 65]
                            sts, pts = [], []
                            for li in range(4):
                                cl = 4 * b + li        # chunk holding l-tile
                                st = stp.tile([128, CH], fp32, tag="st")
                                nc.tensor.matmul(
                                    st[:], kTn[cm][hr, mo:mo + 128],
                                    qTn[cl][hr, :],
                                    start=True, stop=True)
                                sts.append(st)
                                pt = ptp.tile([128, CH], bf16, tag="pt")
                                nc.scalar.activation(pt[:], st[:], AF.Exp)
                                pts.append(pt)
                            for li in range(4):
                                nc.tensor.matmul(
                                    po_l[li][:], vsl, pts[li][:],
                                    start=(m == 0), stop=(m == 15))
                        for li in range(4):
                            rc0 = rcp.tile([1, CH], fp32, tag="rc0")
                            nc.scalar.activation(rc0[:], po_l[li][64:65, :],
                                                 AF.Copy)
                            rc = rcp.tile([1, CH], fp32, tag="rc")
                            nc.vector.reciprocal_approx_fast(rc[:], rc0[:])
                            rcb = rcp.tile([1, CH], bf16, tag="rcb")
                            nc.vector.tensor_copy(rcb[:], rc[:])
                            rb = rbp.tile([64, CH], fp32, tag="rb")
                            nc.tensor.matmul(rb[:], ones64[:], rcb[:],
                                             start=True, stop=True)
                            rbs = rcp.tile([64, CH], fp32, tag="rbs")
                            nc.scalar.activation(rbs[:], rb[:], AF.Copy)
                            ot = otp.tile([64, CH], bf16, tag="ot")
                            nc.vector.tensor_mul(ot[:], po_l[li][0:64, :],
                                                 rbs[:])
                            cj = 4 * b + li
                            nc.sync.dma_start(
                                a2a_in[h][64 * cj:64 * cj + 64, :], ot[:])
                    # h's AllToAll: h=0's overlaps h=1 attention compute
                    nc.gpsimd.collective_compute(
                        "AllToAll", mybir.AluOpType.bypass,
                        replica_groups=[list(range(NC))],
                        ins=[a2a_in[h][:]],
                        outs=[a2a_out[h][:]],
                    )

            if taps:
                nc.sync.dma_start(tap_a2ain[:], a2a_in[0][:])
                nc.sync.dma_start(tap_a2aout[:], a2a_out[0][:])

            # ---------- phase 3: output projection ----------
            with (
                tc.tile_pool(name="ofp", bufs=8) as ofp,
                tc.tile_pool(name="prp", bufs=2, space="PSUM") as prp,
                tc.tile_pool(name="obp", bufs=2) as obp,
            ):
                of = []
                for ff in range(8):
                    t = ofp.tile([128, CH], bf16, tag="of", name=f"of{ff}")
                    nc.sync.dma_start(
                        t[0:64, :], a2a_out[0][64 * ff:64 * (ff + 1), :])
                    nc.sync.dma_start(
                        t[64:128, :], a2a_out[1][64 * ff:64 * (ff + 1), :])
                    of.append(t)
                for d in range(8):
                    pr = prp.tile([128, CH], fp32, tag="pr")
                    for ff in range(8):
                        nc.tensor.matmul(
                            pr[:], wp_sb[ff][:, 128 * d:128 * (d + 1)],
                            of[ff][:], start=(ff == 0), stop=(ff == 7))
                    ob = obp.tile([128, CH], fp32, tag="ob")
                    nc.vector.tensor_scalar_add(ob[:], pr[:],
                                                bp_sb[:, d:d + 1])
                    nc.sync.dma_start(out_d[128 * d:128 * (d + 1), :], ob[:])

    nc.compile()
    return nc


def _run(inputs, trace=False, trace_kwargs=None):
    from concourse.bass_utils import run_bass_kernel_spmd

    if "nc" not in _CACHE:
        _CACHE["nc"] = _build()
    nc = _CACHE["nc"]
    in_maps = _host_inputs(**inputs)
    res = run_bass_kernel_spmd(
        nc, in_maps, core_ids=list(range(NC)), trace=trace,
        **(trace_kwargs or {}))
    return res


def kernel(x, Wqkv, q_scale, k_scale, Wproj, bproj):
    res = _run(dict(x=x, Wqkv=Wqkv, q_scale=q_scale, k_scale=k_scale,
                    Wproj=Wproj, bproj=bproj))
    outT = np.concatenate([res.results[c]["out"] for c in range(NC)], axis=1)
    return np.ascontiguousarray(outT.T).reshape(B, L, DIM).astype(np.float32)


if __name__ == "__main__":
    rng = np.random.default_rng(0)
    x = rng.standard_normal((B, L, DIM), dtype=np.float32)
    Wqkv_ = rng.standard_normal((DIM, 3 * DIM), dtype=np.float32) * DIM ** -0.5
    Wproj_ = rng.standard_normal((DIM, DIM), dtype=np.float32) * DIM ** -0.5
    out = kernel(x=x, Wqkv=Wqkv_, q_scale=np.ones(HD, np.float32),
                 k_scale=np.ones(HD, np.float32), Wproj=Wproj_,
                 bproj=np.zeros(DIM, np.float32))
    print(out.shape, out.dtype)
